# revision 49
# baseline (speedup 1.0000x reference)
"""Trainium2 Bass kernel for a 12-head attention block with cls-token
rebalancing (B=4, N=1024, C=768), distributed over 8 NeuronCores.

Sharding: core = 2*b + g  (b = batch 0..3, g = head-group 0..1, 6 heads each).
Each core computes qkv / attention / softmax / cls-rebalance / attn@v for its
(batch, 6 heads), plus the partial output projection over its heads' channels;
core pairs {2b, 2b+1} AllReduce the projection partials.

Outputs: attn (4,12,1024,1024) fp32 and out (4,1024,768) fp32, matching
reference.py's (out, attn) tuple.
"""

import sys

if "/opt/trn_rl_repo" not in sys.path:
    sys.path.insert(0, "/opt/trn_rl_repo")

from contextlib import ExitStack

import numpy as np

import concourse.bass as bass
import concourse.bacc as bacc
import concourse.tile as tile
from concourse import mybir
from concourse import bass_utils

F32 = mybir.dt.float32
# Matmul payload runs in bf16: fast weight load (FWL), 1 cyc/row, half the DMA.
BF16 = mybir.dt.bfloat16

B, N, C = 4, 1024, 768
H = 12
HPC = 6  # heads per core
HD = 64
SCALE = HD ** -0.5
EPS = 1e-6
NCORES = 8
REPLICA_GROUPS = [[0, 1], [2, 3], [4, 5], [6, 7]]

Exp = mybir.ActivationFunctionType.Exp
ALU = mybir.AluOpType

# bf16 packed-input column offsets (128 partitions)
OFF_xT = 0        # 6 c_in tiles x 1024 n
OFF_wqkT = 6144   # 6 c_in tiles x 768 qk cols
OFF_wvT = 10752   # 6 c_in tiles x 384 v cols
OFF_wpjT = 13056  # 3 c_in tiles x 768 cols
OFF_ones = 15360  # row (partition 0)
OFF_bv = 15488    # row
OFF_bpj = 15872   # row
PACKED = 16640
# fp32 small-constant input offsets
SOFF_ident = 0
SOFF_bqk = 128    # (128, 6)
SOFF_bvc = 134    # (128, 3)
SOFF_cls = 137    # row
SPACKED = 144


def _mm(ap):
    return ap


def build_bass():
    nc = bacc.Bacc("TRN2", debug=False, target_bir_lowering=False, num_devices=NCORES)

    # ---- external I/O: ONE packed input tensor (single DMA -> single
    # semaphore lane, since the PE LDWEIGHTS slot only fits one sync wait) ----
    inp_d = nc.dram_tensor("inp", (128, PACKED), BF16, kind="ExternalInput").ap()
    inps_d = nc.dram_tensor("inps", (128, SPACKED), F32, kind="ExternalInput").ap()

    attn_d = nc.dram_tensor("attn_out", (HPC, N, N), BF16, kind="ExternalOutput").ap()
    out_d = nc.dram_tensor("out_ext", (N // 2, C), BF16, kind="ExternalOutput").ap()
    dbg_d = None  # debug outputs disabled

    # ---- collective bounce buffers ----
    cc_in = nc.dram_tensor("cc_in", (N, C), BF16).ap()
    cc_out = nc.dram_tensor("cc_out", (N // 2, C), BF16).ap()

    with tile.TileContext(nc) as tc, ExitStack() as ctx:
        P = ctx.enter_context  # pool opener

        persist = P(tc.tile_pool(name="persist", bufs=1))
        attn_pool = P(tc.tile_pool(name="attn", bufs=6))
        et_pool = P(tc.tile_pool(name="et", bufs=6))
        bc_pool = P(tc.tile_pool(name="bc", bufs=4))
        out_pool = P(tc.tile_pool(name="outsb", bufs=2))
        ps_pool = P(tc.tile_pool(name="ps", bufs=4, space="PSUM"))
        av_pool = ps_pool  # shared 4-slot rotation (8 psum banks total)

        # ---- persistent SBUF tensors ----
        inp = persist.tile([128, PACKED], BF16, tag="inp")
        inps = persist.tile([128, SPACKED], F32, tag="inps")
        xT = inp[:, OFF_xT:OFF_xT + 6 * N]
        wqkT = inp[:, OFF_wqkT:OFF_wqkT + 6 * 768]
        wvT = inp[:, OFF_wvT:OFF_wvT + 6 * 384]
        wpjT = inp[:, OFF_wpjT:OFF_wpjT + 3 * C]
        ones = inp[0:1, OFF_ones:OFF_ones + 128]
        bv = inp[0:1, OFF_bv:OFF_bv + HPC * HD]
        bpj = inp[0:1, OFF_bpj:OFF_bpj + C]
        ident = inps[:, SOFF_ident:SOFF_ident + 128]
        bqk = inps[:, SOFF_bqk:SOFF_bqk + 6]
        bvc = inps[:, SOFF_bvc:SOFF_bvc + 3]
        clsb = inps[0:1, SOFF_cls:SOFF_cls + HPC]

        qkvT = persist.tile([128, 6 * N], BF16, tag="qkvT")       # m 0..2: q pairs, 3..5: k pairs
        vsb = persist.tile([128, 8 * 384], BF16, tag="vsb")       # 8 n tiles x (6 heads*64)
        outT = persist.tile([128, 3 * N], BF16, tag="outT")       # pair j: c_in x n
        S = persist.tile([128, HPC * 8], F32, tag="S")           # row sums, head h cols h*8..
        iS = persist.tile([128, HPC * 8], F32, tag="iS")         # 1/S
        iSr = persist.tile([1, HPC * N], F32, tag="iSr")         # transposed 1/S rows
        v0T = persist.tile([128, 3], F32, tag="v0T")             # v[0,:] as columns
        cells = persist.tile([1, 8 * HPC], F32, tag="cells")     # per-head scalars
        out_acc = persist.tile([128, 8 * C], F32, tag="out_acc") # proj accumulator
        otiles = [persist.tile([128, C], BF16, name=f"ot{_n}", tag=f"ot{_n}") for _n in range(8)]

        # ---- input DMAs ----
        nc.sync.dma_start(out=inp[:, :], in_=inp_d[:, :])
        nc.sync.dma_start(out=inps[:, :], in_=inps_d[:, :])

        def emit_qkv(m):
            ps = ps_pool.tile([128, N], F32, name=f"qkvps{m}", tag="ps")
            for nh in range(2):
                for k in range(6):
                    nc.tensor.matmul(
                        ps[:, nh * 512:(nh + 1) * 512],
                        _mm(wqkT[:, k * 768 + m * 128: k * 768 + (m + 1) * 128]),
                        _mm(xT[:, k * N + nh * 512: k * N + (nh + 1) * 512]),
                        start=(k == 0), stop=(k == 5),
                    )
            nc.vector.tensor_scalar_add(qkvT[:, m * N:(m + 1) * N], ps[:, :], bqk[:, m:m + 1])

        def emit_v():
            for nt in range(8):
                ps = ps_pool.tile([128, N], F32, name=f"vps{nt}", tag="ps")
                for k in range(6):
                    nc.tensor.matmul(
                        ps[:, 0:384],
                        _mm(xT[:, k * N + nt * 128: k * N + (nt + 1) * 128]),
                        _mm(wvT[:, k * 384:(k + 1) * 384]),
                        start=(k == 0), stop=False,
                    )
                nc.tensor.matmul(ps[:, 0:384], _mm(ones[0:1, :]), _mm(bv[0:1, :]),
                                 start=False, stop=True)
                nc.vector.tensor_copy(vsb[:, nt * 384:(nt + 1) * 384], ps[:, 0:384])
            for mt in range(3):
                ps = ps_pool.tile([128, N], F32, name=f"v0ps{mt}", tag="ps")
                for k in range(6):
                    nc.tensor.matmul(
                        ps[:, 0:1],
                        wvT[:, k * 384 + mt * 128: k * 384 + (mt + 1) * 128],
                        xT[:, k * N: k * N + 1],
                        start=(k == 0), stop=(k == 5),
                    )
                nc.vector.tensor_scalar_add(v0T[:, mt:mt + 1], ps[:, 0:1], bvc[:, mt:mt + 1])

        def emit_pass1(j):
            qt_pair = qkvT[:, j * N:(j + 1) * N]
            kt_pair = qkvT[:, (3 + j) * N:(4 + j) * N]
            for half in range(2):
                h = 2 * j + half
                rows = slice(64 * half, 64 * half + 64)
                for qt in range(8):
                    ps = ps_pool.tile([128, N], F32, name=f"s1_{h}_{qt}", tag="ps")
                    for kh in range(2):
                        nc.tensor.matmul(
                            ps[:, kh * 512:(kh + 1) * 512],
                            _mm(qt_pair[rows, qt * 128:(qt + 1) * 128]),
                            _mm(kt_pair[rows, kh * 512:(kh + 1) * 512]),
                            start=True, stop=True,
                        )
                    at = attn_pool.tile([128, N], BF16, name=f"at{h}_{qt}", tag="attn")
                    sc = S[:, h * 8 + qt: h * 8 + qt + 1]
                    nc.scalar.activation(at[:, :], ps[:, :], Exp, scale=SCALE, accum_out=sc)
                    isc = iS[:, h * 8 + qt: h * 8 + qt + 1]
                    nc.vector.reciprocal(isc, sc)
                    nc.vector.tensor_scalar_mul(at[:, :], at[:, :], isc)

                    if qt == 0:
                        cb = cells[0:1, h * 8: h * 8 + 8]
                        a00 = at[0:1, 0:1]
                        nc.vector.tensor_scalar(cb[0:1, 0:1], a00, clsb[0:1, h:h + 1], 1.0,
                                                op0=ALU.add, op1=ALU.min)
                        nc.vector.tensor_scalar(cb[0:1, 1:2], a00, -1.0, 1.0 + EPS,
                                                op0=ALU.mult, op1=ALU.add)
                        nc.vector.reciprocal(cb[0:1, 2:3], cb[0:1, 1:2])
                        nc.vector.tensor_scalar(cb[0:1, 3:4], cb[0:1, 0:1], -1.0, 1.0,
                                                op0=ALU.mult, op1=ALU.add)
                        nc.vector.tensor_mul(cb[0:1, 4:5], cb[0:1, 3:4], cb[0:1, 2:3])
                        nc.vector.tensor_mul(cb[0:1, 5:6], cb[0:1, 4:5], a00)
                        nc.vector.tensor_sub(cb[0:1, 6:7], cb[0:1, 0:1], cb[0:1, 5:6])
                        nc.vector.tensor_scalar_mul(at[0:1, 1:N], at[0:1, 1:N], cb[0:1, 4:5])
                        nc.vector.tensor_copy(at[0:1, 0:1], cb[0:1, 0:1])

                    nc.sync.dma_start(out=attn_d[h, qt * 128:(qt + 1) * 128, :], in_=at[:, :])

                ps = ps_pool.tile([128, N], F32, name=f"ivt{h}", tag="ps")
                for qt in range(8):
                    nc.tensor.transpose(ps[0:1, qt * 128:(qt + 1) * 128],
                                        iS[:, h * 8 + qt: h * 8 + qt + 1], ident[:, :])
                nc.vector.tensor_copy(iSr[0:1, h * N:(h + 1) * N], ps[0:1, :])

        def emit_pass2(j):
            qt_pair = qkvT[:, j * N:(j + 1) * N]
            kt_pair = qkvT[:, (3 + j) * N:(4 + j) * N]
            avt = [ps_pool.tile([128, N], F32, name=f"avt{j}_{_h}", tag="ps") for _h in range(2)]
            for kt in range(8):
                for half in range(2):
                    h = 2 * j + half
                    rows = slice(64 * half, 64 * half + 64)
                    ps = ps_pool.tile([128, N], F32, name=f"s2_{h}_{kt}", tag="ps")
                    for qh in range(2):
                        nc.tensor.matmul(
                            ps[:, qh * 512:(qh + 1) * 512],
                            _mm(kt_pair[rows, kt * 128:(kt + 1) * 128]),
                            _mm(qt_pair[rows, qh * 512:(qh + 1) * 512]),
                            start=True, stop=True,
                        )
                    et = et_pool.tile([128, N], BF16, name=f"et{h}_{kt}", tag="et")
                    nc.scalar.activation(et[:, :], ps[:, :], Exp, scale=SCALE)
                    vcol = vsb[:, kt * 384 + j * 128: kt * 384 + (j + 1) * 128]
                    for qh in range(2):
                        nc.tensor.matmul(
                            avt[half][:, qh * 512:(qh + 1) * 512],
                            _mm(vcol),
                            _mm(et[:, qh * 512:(qh + 1) * 512]),
                            start=(kt == 0), stop=(kt == 7),
                        )
            for c in range(8):
                for half in range(2):
                    h = 2 * j + half
                    rows = slice(64 * half, 64 * half + 64)
                    bc = bc_pool.tile([128, 128], F32, name=f"bc{half}", tag="bc")
                    nc.gpsimd.partition_broadcast(
                        bc[:, :], iSr[0:1, h * N + c * 128: h * N + (c + 1) * 128])
                    nc.vector.tensor_mul(
                        outT[rows, j * N + c * 128: j * N + (c + 1) * 128],
                        avt[half][rows, c * 128:(c + 1) * 128], bc[rows, :])
            for half in range(2):
                h = 2 * j + half
                rows = slice(64 * half, 64 * half + 64)
                bc = bc_pool.tile([128, 128], F32, name=f"bcf{half}", tag="bc")
                nc.gpsimd.partition_broadcast(bc[:, 0:1], cells[0:1, h * 8 + 4: h * 8 + 5])
                nc.gpsimd.partition_broadcast(bc[:, 1:2], cells[0:1, h * 8 + 6: h * 8 + 7])
                v0 = v0T[rows, j: j + 1]
                col0 = outT[rows, j * N: j * N + 1]
                nc.vector.tensor_scalar_mul(bc[rows, 2:3], v0, bc[rows, 1:2])
                nc.vector.scalar_tensor_tensor(col0, col0, bc[rows, 0:1], bc[rows, 2:3],
                                               op0=ALU.mult, op1=ALU.add)

        # ---- output projection: per-pair contribution accumulated in SBUF
        # right after each pass-2, so only pair 2's slice lands in the tail ----
        def emit_proj_partial(j):
            for nt in range(8):
                ps = ps_pool.tile([128, N], F32, name=f"pj{j}_{nt}", tag="ps")
                for ch in range(2):
                    cs = slice(ch * 512, ch * 512 + 384)
                    nc.tensor.matmul(
                        ps[:, cs],
                        _mm(outT[:, j * N + nt * 128: j * N + (nt + 1) * 128]),
                        _mm(wpjT[:, j * C + ch * 384: j * C + (ch + 1) * 384]),
                        start=True, stop=(j != 0),
                    )
                    if j == 0:
                        nc.tensor.matmul(ps[:, cs], _mm(ones[0:1, :]),
                                         _mm(bpj[0:1, ch * 384:(ch + 1) * 384]),
                                         start=False, stop=True)
                for ch in range(2):
                    cs = slice(ch * 512, ch * 512 + 384)
                    oacc = out_acc[:, nt * C + ch * 384: nt * C + (ch + 1) * 384]
                    if j == 0:
                        nc.vector.tensor_copy(oacc, ps[:, cs])
                    elif j == 1:
                        nc.vector.tensor_add(oacc, oacc, ps[:, cs])
                    else:
                        ot = otiles[nt]
                        nc.vector.tensor_add(ot[:, ch * 384:(ch + 1) * 384], oacc, ps[:, cs])
                if j == 2:
                    nc.sync.dma_start(out=cc_in[nt * 128:(nt + 1) * 128, :], in_=otiles[nt][:, :])

        # emission order: keep ScalarE (exp) continuously fed; v/qkv fill PE
        emit_qkv(0); emit_qkv(3)
        emit_pass1(0)
        emit_qkv(1); emit_qkv(4)
        emit_pass1(1)
        emit_v()
        emit_pass2(0)
        emit_proj_partial(0)
        emit_qkv(2); emit_qkv(5)
        emit_pass1(2)
        emit_pass2(1)
        emit_proj_partial(1)
        emit_pass2(2)
        emit_proj_partial(2)

        # ---- pair ReduceScatter of projection partials: core 2b keeps rows
        # 0:512, core 2b+1 rows 512:1024; host concatenates. ----
        nc.gpsimd.collective_compute(
            "ReduceScatter", ALU.add, replica_groups=REPLICA_GROUPS,
            ins=[cc_in[:, :].opt()], outs=[cc_out[:, :].opt()],
        )
        nc.sync.dma_start(out=out_d[:, :], in_=cc_out[:, :])


    nc.compile()
    _split_waits(nc)
    return nc


def _tiled_cols(a, kk):
    """(kk*128, M) -> (128, kk*M): column block k = rows k*128..(k+1)*128."""
    m = a.shape[1]
    return a.reshape(kk, 128, m).transpose(1, 0, 2).reshape(128, kk * m)


def _split_waits(nc):
    """Walrus codegen caps sync-waits at 1 per instruction (2 for
    EventSemaphore). Spill extra waits onto EventSemaphore NOPs inserted
    just before, on the same engine stream."""
    nid = [0]

    def nop_with(engine, waits):
        nid[0] += 1
        nop = mybir.InstEventSemaphore(name=f"WSPILL-{nid[0]}", ins=[], outs=[])
        nop.engine = engine
        nop.sync_info = mybir.SyncInfo(on_wait=list(waits), on_update=[])
        return nop

    for f in nc.m.functions:
        for blk in f.blocks:
            out = []
            changed = False
            for inst in blk.instructions:
                si = inst.sync_info
                waits = list(si.on_wait) if si is not None and si.on_wait else []
                cap = 2 if isinstance(inst, mybir.InstEventSemaphore) else 1
                if len(waits) > cap:
                    spill, keep = waits[:-cap], waits[-cap:]
                    for i in range(0, len(spill), 2):
                        out.append(nop_with(inst.engine, spill[i:i + 2]))
                    inst.sync_info = mybir.SyncInfo(
                        on_wait=keep, on_update=list(si.on_update) if si.on_update else [])
                    changed = True
                out.append(inst)
            if changed:
                blk.instructions = out


def make_in_maps(x, qkv_w, qkv_b, proj_w, proj_b, cls_bias):
    import ml_dtypes
    f = np.float32
    bf = ml_dtypes.bfloat16
    in_maps = []
    for core in range(NCORES):
        b, g = core // 2, core % 2
        hs = g * HPC
        qrows = qkv_w[hs * HD:(hs + HPC) * HD]            # (384, 768)
        krows = qkv_w[C + hs * HD: C + (hs + HPC) * HD]   # (384, 768)
        vrows = qkv_w[2 * C + hs * HD: 2 * C + (hs + HPC) * HD]
        bq = qkv_b[hs * HD:(hs + HPC) * HD]
        bk = qkv_b[C + hs * HD: C + (hs + HPC) * HD]
        bvv = qkv_b[2 * C + hs * HD: 2 * C + (hs + HPC) * HD]

        packed = np.zeros((128, PACKED), f)
        packed[:, OFF_xT:OFF_xT + 6 * N] = _tiled_cols(np.asarray(x[b]).T.astype(f), 6)
        packed[:, OFF_wqkT:OFF_wqkT + 6 * 768] = _tiled_cols(
            np.concatenate([qrows, krows], 0).T.astype(f), 6)
        packed[:, OFF_wvT:OFF_wvT + 6 * 384] = _tiled_cols(vrows.T.astype(f), 6)
        packed[:, OFF_wpjT:OFF_wpjT + 3 * C] = _tiled_cols(
            np.asarray(proj_w).T[hs * HD:(hs + HPC) * HD, :].astype(f), 3)
        packed[0, OFF_ones:OFF_ones + 128] = 1.0
        packed[0, OFF_bv:OFF_bv + HPC * HD] = bvv
        packed[0, OFF_bpj:OFF_bpj + C] = np.asarray(proj_b) * 0.5

        small = np.zeros((128, SPACKED), f)
        small[:, SOFF_ident:SOFF_ident + 128] = np.eye(128, dtype=f)
        small[:, SOFF_bqk:SOFF_bqk + 6] = np.concatenate([bq, bk]).reshape(6, 128).T
        small[:, SOFF_bvc:SOFF_bvc + 3] = np.asarray(bvv).reshape(3, 128).T
        small[0, SOFF_cls:SOFF_cls + HPC] = cls_bias[hs:hs + HPC]
        in_maps.append({"inp": packed.astype(bf), "inps": small})
    return in_maps


_CACHED_NC = None


def _get_nc():
    global _CACHED_NC
    if _CACHED_NC is None:
        _CACHED_NC = build_bass()
    return _CACHED_NC


def run(trace=False, **inputs):
    nc = _get_nc()
    in_maps = make_in_maps(**inputs)
    res = bass_utils.run_bass_kernel_spmd(
        nc, in_maps, core_ids=list(range(NCORES)), trace=trace,
    )
    attn = np.empty((B, H, N, N), np.float32)
    out = np.empty((B, N, C), np.float32)
    for core in range(NCORES):
        b, g = core // 2, core % 2
        attn[b, g * HPC:(g + 1) * HPC] = np.asarray(res.results[core]["attn_out"], dtype=np.float32)
        out[b, g * (N // 2):(g + 1) * (N // 2)] = np.asarray(res.results[core]["out_ext"], dtype=np.float32)
    return (out, attn), res


def kernel(**inputs):
    outputs, _ = run(trace=False, **inputs)
    return outputs


# revision 52
# speedup vs baseline: 1.1226x; 1.1226x over previous
"""Trainium2 Bass kernel for a 12-head attention block with cls-token
rebalancing (B=4, N=1024, C=768), distributed over 8 NeuronCores.

Sharding: core = 2*b + g  (b = batch 0..3, g = head-group 0..1, 6 heads each).
Each core computes qkv / attention / softmax / cls-rebalance / attn@v for its
(batch, 6 heads), plus the partial output projection over its heads' channels;
core pairs {2b, 2b+1} AllReduce the projection partials.

Outputs: attn (4,12,1024,1024) fp32 and out (4,1024,768) fp32, matching
reference.py's (out, attn) tuple.
"""

import sys

if "/opt/trn_rl_repo" not in sys.path:
    sys.path.insert(0, "/opt/trn_rl_repo")

from contextlib import ExitStack

import numpy as np

import concourse.bass as bass
import concourse.bacc as bacc
import concourse.tile as tile
from concourse import mybir
from concourse import bass_utils

F32 = mybir.dt.float32
# Matmul payload runs in bf16: fast weight load (FWL), 1 cyc/row, half the DMA.
BF16 = mybir.dt.bfloat16

B, N, C = 4, 1024, 768
H = 12
HPC = 6  # heads per core
HD = 64
SCALE = HD ** -0.5
EPS = 1e-6
NCORES = 8
REPLICA_GROUPS = [[0, 1], [2, 3], [4, 5], [6, 7]]

Exp = mybir.ActivationFunctionType.Exp
ALU = mybir.AluOpType

# bf16 packed-input column offsets (128 partitions)
OFF_xT = 0        # 6 c_in tiles x 1024 n
OFF_wqkT = 6144   # 6 c_in tiles x 768 qk cols
OFF_wvT = 10752   # 6 c_in tiles x 384 v cols
OFF_wpjT = 13056  # 3 c_in tiles x 768 cols
OFF_ones = 15360  # row (partition 0)
OFF_bv = 15488    # row
OFF_bpj = 15872   # row
PACKED = 16640
# fp32 small-constant input offsets
SOFF_ident = 0
SOFF_bqk = 128    # (128, 6)
SOFF_bvc = 134    # (128, 3)
SOFF_cls = 137    # row
SPACKED = 144


def _mm(ap):
    return ap


def build_bass():
    nc = bacc.Bacc("TRN2", debug=False, target_bir_lowering=False, num_devices=NCORES)

    # ---- external I/O: ONE packed input tensor (single DMA -> single
    # semaphore lane, since the PE LDWEIGHTS slot only fits one sync wait) ----
    inp_d = nc.dram_tensor("inp", (128, PACKED), BF16, kind="ExternalInput").ap()
    inps_d = nc.dram_tensor("inps", (128, SPACKED), F32, kind="ExternalInput").ap()

    attn_d = nc.dram_tensor("attn_out", (HPC, N, N), BF16, kind="ExternalOutput").ap()
    out_d = nc.dram_tensor("out_ext", (N // 2, C), BF16, kind="ExternalOutput").ap()
    dbg_d = None  # debug outputs disabled

    # ---- collective bounce buffers ----
    cc_in = nc.dram_tensor("cc_in", (N, C), BF16).ap()
    cc_out = nc.dram_tensor("cc_out", (N // 2, C), BF16).ap()

    with tile.TileContext(nc) as tc, ExitStack() as ctx:
        P = ctx.enter_context  # pool opener

        persist = P(tc.tile_pool(name="persist", bufs=1))
        attn_pool = P(tc.tile_pool(name="attn", bufs=6))
        et_pool = P(tc.tile_pool(name="et", bufs=6))
        bc_pool = P(tc.tile_pool(name="bc", bufs=4))
        out_pool = P(tc.tile_pool(name="outsb", bufs=2))
        ps_pool = P(tc.tile_pool(name="ps", bufs=4, space="PSUM"))
        av_pool = ps_pool  # shared 4-slot rotation (8 psum banks total)

        # ---- persistent SBUF tensors ----
        inp = persist.tile([128, PACKED], BF16, tag="inp")
        inps = persist.tile([128, SPACKED], F32, tag="inps")
        xT = inp[:, OFF_xT:OFF_xT + 6 * N]
        wqkT = inp[:, OFF_wqkT:OFF_wqkT + 6 * 768]
        wvT = inp[:, OFF_wvT:OFF_wvT + 6 * 384]
        wpjT = inp[:, OFF_wpjT:OFF_wpjT + 3 * C]
        ones = inp[0:1, OFF_ones:OFF_ones + 128]
        bv = inp[0:1, OFF_bv:OFF_bv + HPC * HD]
        bpj = inp[0:1, OFF_bpj:OFF_bpj + C]
        ident = inps[:, SOFF_ident:SOFF_ident + 128]
        bqk = inps[:, SOFF_bqk:SOFF_bqk + 6]
        bvc = inps[:, SOFF_bvc:SOFF_bvc + 3]
        clsb = inps[0:1, SOFF_cls:SOFF_cls + HPC]

        qkvT = persist.tile([128, 6 * N], BF16, tag="qkvT")       # m 0..2: q pairs, 3..5: k pairs
        vsb = persist.tile([128, 8 * 384], BF16, tag="vsb")       # 8 n tiles x (6 heads*64)
        outT = persist.tile([128, 3 * N], BF16, tag="outT")       # pair j: c_in x n
        S = persist.tile([128, HPC * 8], F32, tag="S")           # row sums, head h cols h*8..
        iS = persist.tile([128, HPC * 8], F32, tag="iS")         # 1/S
        iSr = persist.tile([1, HPC * N], F32, tag="iSr")         # transposed 1/S rows
        v0T = persist.tile([128, 3], F32, tag="v0T")             # v[0,:] as columns
        cells = persist.tile([1, 8 * HPC], F32, tag="cells")     # per-head scalars

        # ---- input DMAs ----
        nc.sync.dma_start(out=inp[:, :], in_=inp_d[:, :])
        nc.sync.dma_start(out=inps[:, :], in_=inps_d[:, :])

        def emit_qkv(m):
            ps = ps_pool.tile([128, N], F32, name=f"qkvps{m}", tag="ps")
            for nh in range(2):
                for k in range(6):
                    nc.tensor.matmul(
                        ps[:, nh * 512:(nh + 1) * 512],
                        _mm(wqkT[:, k * 768 + m * 128: k * 768 + (m + 1) * 128]),
                        _mm(xT[:, k * N + nh * 512: k * N + (nh + 1) * 512]),
                        start=(k == 0), stop=(k == 5),
                    )
            nc.vector.tensor_scalar_add(qkvT[:, m * N:(m + 1) * N], ps[:, :], bqk[:, m:m + 1])

        def emit_v():
            for nt in range(8):
                ps = ps_pool.tile([128, N], F32, name=f"vps{nt}", tag="ps")
                for k in range(6):
                    nc.tensor.matmul(
                        ps[:, 0:384],
                        _mm(xT[:, k * N + nt * 128: k * N + (nt + 1) * 128]),
                        _mm(wvT[:, k * 384:(k + 1) * 384]),
                        start=(k == 0), stop=False,
                    )
                nc.tensor.matmul(ps[:, 0:384], _mm(ones[0:1, :]), _mm(bv[0:1, :]),
                                 start=False, stop=True)
                nc.vector.tensor_copy(vsb[:, nt * 384:(nt + 1) * 384], ps[:, 0:384])
            for mt in range(3):
                ps = ps_pool.tile([128, N], F32, name=f"v0ps{mt}", tag="ps")
                for k in range(6):
                    nc.tensor.matmul(
                        ps[:, 0:1],
                        wvT[:, k * 384 + mt * 128: k * 384 + (mt + 1) * 128],
                        xT[:, k * N: k * N + 1],
                        start=(k == 0), stop=(k == 5),
                    )
                nc.vector.tensor_scalar_add(v0T[:, mt:mt + 1], ps[:, 0:1], bvc[:, mt:mt + 1])

        def emit_pass1(j):
            qt_pair = qkvT[:, j * N:(j + 1) * N]
            kt_pair = qkvT[:, (3 + j) * N:(4 + j) * N]
            for half in range(2):
                h = 2 * j + half
                rows = slice(64 * half, 64 * half + 64)
                for qt in range(8):
                    ps = ps_pool.tile([128, N], F32, name=f"s1_{h}_{qt}", tag="ps")
                    for kh in range(2):
                        nc.tensor.matmul(
                            ps[:, kh * 512:(kh + 1) * 512],
                            _mm(qt_pair[rows, qt * 128:(qt + 1) * 128]),
                            _mm(kt_pair[rows, kh * 512:(kh + 1) * 512]),
                            start=True, stop=True,
                        )
                    at = attn_pool.tile([128, N], BF16, name=f"at{h}_{qt}", tag="attn")
                    sc = S[:, h * 8 + qt: h * 8 + qt + 1]
                    nc.scalar.activation(at[:, :], ps[:, :], Exp, scale=SCALE, accum_out=sc)
                    isc = iS[:, h * 8 + qt: h * 8 + qt + 1]
                    nc.vector.reciprocal(isc, sc)
                    nc.vector.tensor_scalar_mul(at[:, :], at[:, :], isc)

                    if qt == 0:
                        cb = cells[0:1, h * 8: h * 8 + 8]
                        a00 = at[0:1, 0:1]
                        nc.vector.tensor_scalar(cb[0:1, 0:1], a00, clsb[0:1, h:h + 1], 1.0,
                                                op0=ALU.add, op1=ALU.min)
                        nc.vector.tensor_scalar(cb[0:1, 1:2], a00, -1.0, 1.0 + EPS,
                                                op0=ALU.mult, op1=ALU.add)
                        nc.vector.reciprocal(cb[0:1, 2:3], cb[0:1, 1:2])
                        nc.vector.tensor_scalar(cb[0:1, 3:4], cb[0:1, 0:1], -1.0, 1.0,
                                                op0=ALU.mult, op1=ALU.add)
                        nc.vector.tensor_mul(cb[0:1, 4:5], cb[0:1, 3:4], cb[0:1, 2:3])
                        nc.vector.tensor_mul(cb[0:1, 5:6], cb[0:1, 4:5], a00)
                        nc.vector.tensor_sub(cb[0:1, 6:7], cb[0:1, 0:1], cb[0:1, 5:6])
                        nc.vector.tensor_scalar_mul(at[0:1, 1:N], at[0:1, 1:N], cb[0:1, 4:5])
                        nc.vector.tensor_copy(at[0:1, 0:1], cb[0:1, 0:1])

                    nc.sync.dma_start(out=attn_d[h, qt * 128:(qt + 1) * 128, :], in_=at[:, :])

                ps = ps_pool.tile([128, N], F32, name=f"ivt{h}", tag="ps")
                for qt in range(8):
                    nc.tensor.transpose(ps[0:1, qt * 128:(qt + 1) * 128],
                                        iS[:, h * 8 + qt: h * 8 + qt + 1], ident[:, :])
                nc.vector.tensor_copy(iSr[0:1, h * N:(h + 1) * N], ps[0:1, :])

        def emit_pass2(j):
            qt_pair = qkvT[:, j * N:(j + 1) * N]
            kt_pair = qkvT[:, (3 + j) * N:(4 + j) * N]
            avt = [ps_pool.tile([128, N], F32, name=f"avt{j}_{_h}", tag="ps") for _h in range(2)]
            for kt in range(8):
                for half in range(2):
                    h = 2 * j + half
                    rows = slice(64 * half, 64 * half + 64)
                    ps = ps_pool.tile([128, N], F32, name=f"s2_{h}_{kt}", tag="ps")
                    for qh in range(2):
                        nc.tensor.matmul(
                            ps[:, qh * 512:(qh + 1) * 512],
                            _mm(kt_pair[rows, kt * 128:(kt + 1) * 128]),
                            _mm(qt_pair[rows, qh * 512:(qh + 1) * 512]),
                            start=True, stop=True,
                        )
                    et = et_pool.tile([128, N], BF16, name=f"et{h}_{kt}", tag="et")
                    nc.scalar.activation(et[:, :], ps[:, :], Exp, scale=SCALE)
                    vcol = vsb[:, kt * 384 + j * 128: kt * 384 + (j + 1) * 128]
                    for qh in range(2):
                        nc.tensor.matmul(
                            avt[half][:, qh * 512:(qh + 1) * 512],
                            _mm(vcol),
                            _mm(et[:, qh * 512:(qh + 1) * 512]),
                            start=(kt == 0), stop=(kt == 7),
                        )
            for c in range(8):
                for half in range(2):
                    h = 2 * j + half
                    rows = slice(64 * half, 64 * half + 64)
                    bc = bc_pool.tile([128, 128], F32, name=f"bc{half}", tag="bc")
                    nc.gpsimd.partition_broadcast(
                        bc[:, :], iSr[0:1, h * N + c * 128: h * N + (c + 1) * 128])
                    nc.vector.tensor_mul(
                        outT[rows, j * N + c * 128: j * N + (c + 1) * 128],
                        avt[half][rows, c * 128:(c + 1) * 128], bc[rows, :])
            for half in range(2):
                h = 2 * j + half
                rows = slice(64 * half, 64 * half + 64)
                bc = bc_pool.tile([128, 128], F32, name=f"bcf{half}", tag="bc")
                nc.gpsimd.partition_broadcast(bc[:, 0:1], cells[0:1, h * 8 + 4: h * 8 + 5])
                nc.gpsimd.partition_broadcast(bc[:, 1:2], cells[0:1, h * 8 + 6: h * 8 + 7])
                v0 = v0T[rows, j: j + 1]
                col0 = outT[rows, j * N: j * N + 1]
                nc.vector.tensor_scalar_mul(bc[rows, 2:3], v0, bc[rows, 1:2])
                nc.vector.scalar_tensor_tensor(col0, col0, bc[rows, 0:1], bc[rows, 2:3],
                                               op0=ALU.mult, op1=ALU.add)

        # emission order: keep ScalarE (exp) continuously fed; v/qkv fill PE
        emit_qkv(0); emit_qkv(3)
        emit_pass1(0)
        emit_qkv(1); emit_qkv(4)
        emit_pass1(1)
        emit_v()
        emit_pass2(0)
        emit_qkv(2); emit_qkv(5)
        emit_pass1(2)
        emit_pass2(1)
        emit_pass2(2)

        # ---- output projection partial: out_part[n, c] over this group's c_in ----
        for nt in range(8):
            ps = ps_pool.tile([128, N], F32, tag="ps")
            for ch in range(2):
                # bank-aligned regions: [0:384] in bank 0, [512:896] in bank 1
                cs = slice(ch * 512, ch * 512 + 384)
                for ktj in range(3):
                    nc.tensor.matmul(
                        ps[:, cs],
                        _mm(outT[:, ktj * N + nt * 128: ktj * N + (nt + 1) * 128]),
                        _mm(wpjT[:, ktj * C + ch * 384: ktj * C + (ch + 1) * 384]),
                        start=(ktj == 0), stop=False,
                    )
                nc.tensor.matmul(ps[:, cs], _mm(ones[0:1, :]),
                                 _mm(bpj[0:1, ch * 384:(ch + 1) * 384]),
                                 start=False, stop=True)
            ot = out_pool.tile([128, C], BF16, tag="outsb")
            nc.vector.tensor_copy(ot[:, 0:384], ps[:, 0:384])
            nc.vector.tensor_copy(ot[:, 384:768], ps[:, 512:896])
            nc.sync.dma_start(out=cc_in[nt * 128:(nt + 1) * 128, :], in_=ot[:, :])

        # ---- pair ReduceScatter of projection partials: core 2b keeps rows
        # 0:512, core 2b+1 rows 512:1024; host concatenates. ----
        nc.gpsimd.collective_compute(
            "ReduceScatter", ALU.add, replica_groups=REPLICA_GROUPS,
            ins=[cc_in[:, :].opt()], outs=[cc_out[:, :].opt()],
        )
        nc.sync.dma_start(out=out_d[:, :], in_=cc_out[:, :])


    nc.compile()
    _split_waits(nc)
    return nc


def _tiled_cols(a, kk):
    """(kk*128, M) -> (128, kk*M): column block k = rows k*128..(k+1)*128."""
    m = a.shape[1]
    return a.reshape(kk, 128, m).transpose(1, 0, 2).reshape(128, kk * m)


def _split_waits(nc):
    """Walrus codegen caps sync-waits at 1 per instruction (2 for
    EventSemaphore). Spill extra waits onto EventSemaphore NOPs inserted
    just before, on the same engine stream."""
    nid = [0]

    def nop_with(engine, waits):
        nid[0] += 1
        nop = mybir.InstEventSemaphore(name=f"WSPILL-{nid[0]}", ins=[], outs=[])
        nop.engine = engine
        nop.sync_info = mybir.SyncInfo(on_wait=list(waits), on_update=[])
        return nop

    for f in nc.m.functions:
        for blk in f.blocks:
            out = []
            changed = False
            for inst in blk.instructions:
                si = inst.sync_info
                waits = list(si.on_wait) if si is not None and si.on_wait else []
                cap = 2 if isinstance(inst, mybir.InstEventSemaphore) else 1
                if len(waits) > cap:
                    spill, keep = waits[:-cap], waits[-cap:]
                    for i in range(0, len(spill), 2):
                        out.append(nop_with(inst.engine, spill[i:i + 2]))
                    inst.sync_info = mybir.SyncInfo(
                        on_wait=keep, on_update=list(si.on_update) if si.on_update else [])
                    changed = True
                out.append(inst)
            if changed:
                blk.instructions = out


def make_in_maps(x, qkv_w, qkv_b, proj_w, proj_b, cls_bias):
    import ml_dtypes
    f = np.float32
    bf = ml_dtypes.bfloat16
    in_maps = []
    for core in range(NCORES):
        b, g = core // 2, core % 2
        hs = g * HPC
        qrows = qkv_w[hs * HD:(hs + HPC) * HD]            # (384, 768)
        krows = qkv_w[C + hs * HD: C + (hs + HPC) * HD]   # (384, 768)
        vrows = qkv_w[2 * C + hs * HD: 2 * C + (hs + HPC) * HD]
        bq = qkv_b[hs * HD:(hs + HPC) * HD]
        bk = qkv_b[C + hs * HD: C + (hs + HPC) * HD]
        bvv = qkv_b[2 * C + hs * HD: 2 * C + (hs + HPC) * HD]

        packed = np.zeros((128, PACKED), f)
        packed[:, OFF_xT:OFF_xT + 6 * N] = _tiled_cols(np.asarray(x[b]).T.astype(f), 6)
        packed[:, OFF_wqkT:OFF_wqkT + 6 * 768] = _tiled_cols(
            np.concatenate([qrows, krows], 0).T.astype(f), 6)
        packed[:, OFF_wvT:OFF_wvT + 6 * 384] = _tiled_cols(vrows.T.astype(f), 6)
        packed[:, OFF_wpjT:OFF_wpjT + 3 * C] = _tiled_cols(
            np.asarray(proj_w).T[hs * HD:(hs + HPC) * HD, :].astype(f), 3)
        packed[0, OFF_ones:OFF_ones + 128] = 1.0
        packed[0, OFF_bv:OFF_bv + HPC * HD] = bvv
        packed[0, OFF_bpj:OFF_bpj + C] = np.asarray(proj_b) * 0.5

        small = np.zeros((128, SPACKED), f)
        small[:, SOFF_ident:SOFF_ident + 128] = np.eye(128, dtype=f)
        small[:, SOFF_bqk:SOFF_bqk + 6] = np.concatenate([bq, bk]).reshape(6, 128).T
        small[:, SOFF_bvc:SOFF_bvc + 3] = np.asarray(bvv).reshape(3, 128).T
        small[0, SOFF_cls:SOFF_cls + HPC] = cls_bias[hs:hs + HPC]
        in_maps.append({"inp": packed.astype(bf), "inps": small})
    return in_maps


_CACHED_NC = None


def _get_nc():
    global _CACHED_NC
    if _CACHED_NC is None:
        _CACHED_NC = build_bass()
    return _CACHED_NC


def run(trace=False, **inputs):
    nc = _get_nc()
    in_maps = make_in_maps(**inputs)
    res = bass_utils.run_bass_kernel_spmd(
        nc, in_maps, core_ids=list(range(NCORES)), trace=trace,
    )
    attn = np.empty((B, H, N, N), np.float32)
    out = np.empty((B, N, C), np.float32)
    for core in range(NCORES):
        b, g = core // 2, core % 2
        attn[b, g * HPC:(g + 1) * HPC] = np.asarray(res.results[core]["attn_out"], dtype=np.float32)
        out[b, g * (N // 2):(g + 1) * (N // 2)] = np.asarray(res.results[core]["out_ext"], dtype=np.float32)
    return (out, attn), res


def kernel(**inputs):
    outputs, _ = run(trace=False, **inputs)
    return outputs
